# revision 48
# baseline (speedup 1.0000x reference)
"""Trainium2 Bass kernel for topk_masking (L2Prompt-style prompted aggregation).

Computes, for ppg [B,1,D], keys/prompt [P,D], k:
  cos   = cosine_similarity(ppg[:,0,:], keys)          [B,P]
  score = 1 - cos; top-k smallest scores per row
  out   = (ppg + 0.5 * sum of the k selected prompt rows,
           sum of selected scores, entropy of softmax(score) summed over B,P)

Sharding: data-parallel over batch B across 8 NeuronCores; keys/prompt
replicated. Scalar partials reduced on host.

Per-core pipeline (BL=1024 rows, 8 b-tiles of 128):
  phase A   keys -> keysT = keys^T/||keys|| via PE transposes (DVE row-scale)
  s1a(bt)   load ppg tile, row norms, PE-transpose raw q -> qT
  s1b(bt)   cos matmul (contract D, f32), PSUM->SBUF scaled by 1/||q|| (DVE)
  stage2(bt) max8 top-k, exp/entropy partials, threshold mask (in-place),
            PE-transpose mask -> maskT, agg matmul maskT^T @ prompt (f32r),
            prompted = ppg + 0.5*agg (in-place), DMA out
  s1a/s1b run 2/1 b-tiles ahead so PE stays busy during the stats chain.

Self-contained: hardcodes shapes B=8192, P=1024, D=2048, 8 cores.
"""

import os
import sys

import numpy as np

for _p in ("/opt/trn_rl_repo",):
    if _p not in sys.path and os.path.isdir(_p):
        sys.path.insert(0, _p)

import concourse.mybir as mybir
from concourse import bacc as bacc_mod
from concourse import bass_utils
from concourse.masks import make_identity
from concourse.tile import TileContext

B, P, D = 8192, 1024, 2048
NCORES = 8
BL = B // NCORES          # rows per core = 1024
BT = BL // 128            # b-tiles per core = 8
PT = P // 128             # p-tiles = 8
DT = D // 128             # d-blocks = 16

F32 = mybir.dt.float32
F32R = mybir.dt.float32r
BF16 = mybir.dt.bfloat16

COS_F32R = os.environ.get("KERNEL_COS_F32R", "0") == "1"
AGG_F32R = os.environ.get("KERNEL_AGG_F32R", "1") == "1"


def build_kernel(k=5, cos_f32r=False, agg_f32r=True):
    """Build the per-core Bass program (identical on all 8 cores)."""
    assert 1 <= k <= 8
    nc = bacc_mod.Bacc(trn_type="TRN2")

    ppg_d = nc.dram_tensor("ppg", [BL, D], F32, kind="ExternalInput")
    keys_d = nc.dram_tensor("keys", [P, D], F32, kind="ExternalInput")
    prompt_d = nc.dram_tensor("prompt", [P, D], F32, kind="ExternalInput")
    out_d = nc.dram_tensor("out", [BL, D], F32, kind="ExternalOutput")
    # stats[:, j]   = sum of top-k cos for b-tile j's rows
    # stats[:, 8+j] = per-row entropy for b-tile j's rows
    stats_d = nc.dram_tensor("stats", [128, 16], F32, kind="ExternalOutput")

    cos_dt = F32R if cos_f32r else F32
    agg_dt = F32R if agg_f32r else F32

    with TileContext(nc) as tc:
        with (
            tc.tile_pool(name="singles", bufs=1) as singles,
            tc.tile_pool(name="rows", bufs=4) as rows,
            tc.tile_pool(name="big", bufs=2) as big,
            tc.tile_pool(name="mid", bufs=2) as mid,
            tc.tile_pool(name="small", bufs=3) as small,
            tc.tile_pool(name="scratch", bufs=3) as scratch,
            tc.tile_pool(name="mtpool", bufs=1) as mtpool,
            tc.tile_pool(name="psum_mm", bufs=4, space="PSUM") as psum_mm,
            tc.tile_pool(name="psum_tr", bufs=2, space="PSUM") as psum_tr,
            tc.tile_pool(name="psum_tt", bufs=1, space="PSUM") as psum_tt,
        ):
            identity = singles.tile([128, 128], F32)
            make_identity(nc, identity)

            # prompt resident, natural [P, D] layout as [128, PT, D].
            # DMA'd per p-tile, emitted late (first needed by stage2(0)'s agg
            # matmul ~150us in) so it doesn't delay keys/ppg loads.
            prompt_sb = singles.tile([128, PT, D], agg_dt)
            prompt_src = prompt_d[:, :].rearrange("(n p) d -> p n d", p=128)
            if agg_f32r:
                prompt_src = prompt_src.bitcast(F32R)

            def load_prompt(pblk):
                nc.sync.dma_start(
                    prompt_sb[:, pblk, :], prompt_src[:, pblk, :]
                )

            # keysT resident: [128 (d within block), DT, P] = keys^T / kn,
            # split into bf16 hi + lo so the cos matmul can run as
            # hi*hi + hi*lo + lo*hi at bf16 speed with ~fp32 accuracy
            keysT_hi = singles.tile([128, DT, P], BF16)
            keysT_lo = singles.tile([128, DT, P], BF16)

            stats_sb = singles.tile([128, 16], F32)
            s_all = singles.tile([128, BT], F32)   # sum exp(-cos) per row
            u_all = singles.tile([128, BT], F32)   # sum cos*exp(-cos) per row

            # per-bt state carried between pipeline stages
            qnr_tiles = [None] * BT
            qT_tiles = [None] * BT
            cos_tiles = [None] * BT

            def phase_a_chunk(chunk):
                """Two keys p-tiles -> normalized bf16 hi/lo keysT columns."""
                khat = []
                for j in range(2):
                    pt = chunk * 2 + j
                    ktile = rows.tile([128, D], F32, tag="rowtile")
                    nc.sync.dma_start(ktile, keys_d[pt * 128 : (pt + 1) * 128, :])
                    sq = scratch.tile([128, D], BF16, tag="bfscr")
                    kn2 = small.tile([128, 1], F32, tag="norm2")
                    nc.scalar.activation(
                        out=sq, in_=ktile,
                        func=mybir.ActivationFunctionType.Square,
                        accum_out=kn2,
                    )
                    kn = small.tile([128, 1], F32, tag="norm")
                    nc.scalar.sqrt(kn, kn2)
                    knr = small.tile([128, 1], F32, tag="nrecip")
                    nc.vector.reciprocal(knr, kn)
                    khat_t = rows.tile([128, D], F32, tag="rowtile")
                    nc.vector.tensor_scalar(
                        khat_t, ktile, knr, None, op0=mybir.AluOpType.mult
                    )
                    khat.append(khat_t)
                for dblk in range(DT):
                    pst = psum_tr.tile([128, 256], F32, tag="tr")
                    for j in range(2):
                        nc.tensor.transpose(
                            pst[:, j * 128 : (j + 1) * 128],
                            khat[j][:, dblk * 128 : (dblk + 1) * 128],
                            identity,
                        )
                    hi = keysT_hi[:, dblk, chunk * 256 : (chunk + 1) * 256]
                    nc.vector.tensor_copy(hi, pst)
                    nc.vector.tensor_sub(
                        keysT_lo[:, dblk, chunk * 256 : (chunk + 1) * 256],
                        pst, hi,
                    )

            def s1a(bt):
                """Load ppg tile, row norms, transpose raw q -> qT."""
                ppg_t = rows.tile([128, D], F32, tag="rowtile")
                nc.sync.dma_start(ppg_t, ppg_d[bt * 128 : (bt + 1) * 128, :])
                sq = scratch.tile([128, D], BF16, tag="bfscr")
                qn2 = small.tile([128, 1], F32, tag="norm2")
                nc.scalar.activation(
                    out=sq, in_=ppg_t,
                    func=mybir.ActivationFunctionType.Square,
                    accum_out=qn2,
                )
                qn = small.tile([128, 1], F32, tag="norm")
                nc.scalar.sqrt(qn, qn2)
                qnr = small.tile([128, 1], F32, tag="nrecip")
                nc.vector.reciprocal(qnr, qn)

                qT_hi = big.tile([128, DT, 128], BF16, tag="qT_hi")
                qT_lo = big.tile([128, DT, 128], BF16, tag="qT_lo")
                for dgrp in range(DT // 4):
                    pst = psum_tr.tile([128, 512], F32, tag="tr")
                    for j in range(4):
                        dblk = dgrp * 4 + j
                        nc.tensor.transpose(
                            pst[:, j * 128 : (j + 1) * 128],
                            ppg_t[:, dblk * 128 : (dblk + 1) * 128],
                            identity,
                        )
                    hi = qT_hi[:, dgrp * 4 : (dgrp + 1) * 4, :]
                    nc.vector.tensor_copy(hi, pst)
                    nc.vector.tensor_sub(
                        qT_lo[:, dgrp * 4 : (dgrp + 1) * 4, :], pst, hi
                    )
                qnr_tiles[bt], qT_tiles[bt] = qnr, (qT_hi, qT_lo)

            def s1b(bt):
                """cos matmul + PSUM->SBUF row-scaled copy (DVE)."""
                (qT_hi, qT_lo), qnr = qT_tiles[bt], qnr_tiles[bt]
                cos_sb = mid.tile([128, P], F32, tag="cos")
                passes = (
                    (qT_hi, keysT_hi), (qT_hi, keysT_lo), (qT_lo, keysT_hi),
                )
                for half in range(2):
                    psc = psum_mm.tile([128, 512], F32, tag="mm")
                    for pi, (lhs, rhs) in enumerate(passes):
                        for kt in range(DT):
                            nc.tensor.matmul(
                                psc,
                                lhs[:, kt, :],
                                rhs[:, kt, half * 512 : (half + 1) * 512],
                                start=(pi == 0 and kt == 0),
                                stop=(pi == 2 and kt == DT - 1),
                            )
                    nc.vector.tensor_scalar(
                        cos_sb[:, half * 512 : (half + 1) * 512],
                        psc, qnr, None, op0=mybir.AluOpType.mult,
                    )
                cos_tiles[bt] = cos_sb

            def stage2(bt):
                """top-k stats, entropy partials, mask, agg, output."""
                cos_sb = cos_tiles[bt]
                # re-load ppg for the final add (cheaper than pinning the
                # s1a tile across the whole pipeline: DMA has headroom)
                ppg_t = rows.tile([128, D], F32, tag="rowtile")
                nc.sync.dma_start(ppg_t, ppg_d[bt * 128 : (bt + 1) * 128, :])
                v8 = small.tile([128, 8], F32, tag="v8")
                nc.vector.max(out=v8, in_=cos_sb)
                nc.vector.tensor_reduce(
                    stats_sb[:, bt : bt + 1], v8[:, 0:k],
                    axis=mybir.AxisListType.X, op=mybir.AluOpType.add,
                )

                # p_i = exp(-cos_i); s = sum_i p_i   (|cos|<=1: no max-shift)
                p_t = mid.tile([128, P], BF16, tag="p")
                nc.scalar.activation(
                    out=p_t, in_=cos_sb,
                    func=mybir.ActivationFunctionType.Exp,
                    scale=-1.0, accum_out=s_all[:, bt : bt + 1],
                )
                # u2 = sum_i p_i * cos_i  (H_row = log s + u2/s)
                # (shares the 4KB/partition "sq" scratch slot; value unused)
                tt_out = psum_tt.tile([128, P], F32, tag="tt")
                nc.vector.scalar_tensor_tensor(
                    out=tt_out, in0=p_t, scalar=1.0, in1=cos_sb,
                    op0=mybir.AluOpType.mult, op1=mybir.AluOpType.mult,
                    accum_out=u_all[:, bt : bt + 1],
                )

                # mask = cos >= kth largest, in place over cos_sb (all other
                # readers of cos are sequenced before this write)
                thr = v8[:, k - 1 : k]
                nc.vector.tensor_scalar(
                    cos_sb, cos_sb, thr, None, op0=mybir.AluOpType.is_ge
                )

                # maskT: [128 (p in block), PT, 128 (b)]
                maskT = mtpool.tile([128, PT, 128], agg_dt, tag="maskT")
                for pgrp in range(PT // 4):
                    pst = psum_tr.tile([128, 512], F32, tag="tr")
                    for j in range(4):
                        pblk = pgrp * 4 + j
                        nc.tensor.transpose(
                            pst[:, j * 128 : (j + 1) * 128],
                            cos_sb[:, pblk * 128 : (pblk + 1) * 128],
                            identity,
                        )
                    nc.vector.tensor_copy(
                        maskT[:, pgrp * 4 : (pgrp + 1) * 4, :], pst
                    )

                # agg = maskT^T @ prompt; prompted = ppg + 0.5*agg (in place)
                for nd in range(D // 512):
                    psa = psum_mm.tile([128, 512], F32, tag="mm")
                    for pblk in range(PT):
                        nc.tensor.matmul(
                            psa,
                            maskT[:, pblk, :],
                            prompt_sb[:, pblk, nd * 512 : (nd + 1) * 512],
                            start=(pblk == 0),
                            stop=(pblk == PT - 1),
                        )
                    nc.vector.scalar_tensor_tensor(
                        out=ppg_t[:, nd * 512 : (nd + 1) * 512],
                        in0=psa, scalar=0.5,
                        in1=ppg_t[:, nd * 512 : (nd + 1) * 512],
                        op0=mybir.AluOpType.mult, op1=mybir.AluOpType.add,
                    )
                nc.sync.dma_start(out_d[bt * 128 : (bt + 1) * 128, :], ppg_t)

            # ---- emission: phase A interleaved with early s1a, then the
            # ---- software-pipelined b-tile loop (s1a 2 ahead, s1b 1 ahead)
            for chunk in range(PT // 2):
                phase_a_chunk(chunk)
                if chunk < 2:
                    s1a(chunk)  # bt 0,1 early
                load_prompt(2 * chunk)
                load_prompt(2 * chunk + 1)
            s1b(0)
            for bt in range(BT):
                if bt + 2 < BT:
                    s1a(bt + 2)
                if bt + 1 < BT:
                    s1b(bt + 1)
                stage2(bt)

            # batched entropy epilogue: H_row = log s + u2/s
            sinv = small.tile([128, BT], F32, tag="sinv8")
            nc.vector.reciprocal(sinv, s_all)
            logs = small.tile([128, BT], F32, tag="logs8")
            nc.scalar.activation(
                out=logs, in_=s_all, func=mybir.ActivationFunctionType.Ln
            )
            usv = small.tile([128, BT], F32, tag="usv8")
            nc.vector.tensor_mul(usv, u_all, sinv)
            nc.vector.tensor_add(stats_sb[:, 8:16], logs, usv)

            nc.sync.dma_start(stats_d[:, :], stats_sb)

    nc.finalize()
    return nc


_CACHE = {}


def _get_nc(k):
    key = (int(k), COS_F32R, AGG_F32R)
    if key not in _CACHE:
        _CACHE[key] = build_kernel(k=key[0], cos_f32r=key[1], agg_f32r=key[2])
    return _CACHE[key]


_LAST_RESULTS = {}


def run_on_cores(inputs, k, trace=False):
    """Run the SPMD kernel on 8 cores; returns BassKernelResults."""
    nc = _get_nc(k)
    ppg = np.ascontiguousarray(
        np.asarray(inputs["ppg"], dtype=np.float32).reshape(B, D)
    )
    keys = np.ascontiguousarray(np.asarray(inputs["keys"], dtype=np.float32))
    prompt = np.ascontiguousarray(np.asarray(inputs["prompt"], dtype=np.float32))
    in_maps = [
        {
            "ppg": ppg[c * BL : (c + 1) * BL],
            "keys": keys,
            "prompt": prompt,
        }
        for c in range(NCORES)
    ]
    res = bass_utils.run_bass_kernel_spmd(
        nc, in_maps, core_ids=list(range(NCORES)), trace=trace
    )
    _LAST_RESULTS["res"] = res
    return res


def kernel(ppg, keys, prompt, k):
    k = int(k)
    inputs = {"ppg": ppg, "keys": keys, "prompt": prompt}
    res = run_on_cores(inputs, k, trace=False)
    outs = res.results
    prompted = np.concatenate([m["out"] for m in outs], axis=0).reshape(B, 1, D)
    topk_cos = np.sum(
        np.stack([m["stats"][:, :8] for m in outs]).astype(np.float64)
    )
    ent = np.sum(np.stack([m["stats"][:, 8:] for m in outs]).astype(np.float64))
    score_sum = np.float32(k * B - topk_cos)
    entropy = np.float32(ent)
    return prompted, score_sum, entropy
